# revision 86
# baseline (speedup 1.0000x reference)
"""MoE expert-FFN kernel for Trainium2, expert-parallel across 8 NeuronCores.

Problem: out[t] = silu(x[t] @ W1[e_t]^T) @ W2[e_t]^T with
  E=64 experts, D=512, H=1024, T=256 tokens.

Memory-bound on expert weights. Strategy:
  - Core c owns experts [8c, 8c+8); host routes tokens (all-to-all on host),
    padding each expert's tokens to capacity C (=32 here).
  - Weights are stored in HBM as fp8 e3m4 with per-channel scales that never
    touch the device: W1's per-d scales are folded into the packed x copies,
    W2's per-d scales into host output unpacking. 8.4MB/core vs 33.5MB fp32.
  - Matmuls run mixed-precision (bf16 stationary activations x fp8e3 moving
    weights, fp32 PSUM), verified exact on HW including fp8 subnormals.
  - Four 32-token expert blocks are stacked on PSUM partitions (offsets
    0/32/64/96), engaging 128x32 column tiling so the four blocks' weight
    streams can overlap in the PE array. One ACT Silu per 4-expert quad,
    8 batched [128,128] PE transposes, then fc2 back to [tok, D].
"""

import numpy as np

E, D, H, T = 64, 512, 1024, 256
NCORES = 8
EPC = E // NCORES          # experts per core
DC = D // 128              # 4 d-chunks
HC = H // 128              # 8 h-chunks
CB = 32                    # token block (PSUM partition-stacking granularity)
QUAD = 4                   # token blocks per PSUM quad

WQCOLS = 8192              # per-expert weight columns (W1 4096 | W2 4096)
FP8_MAX = 15.0             # e3m4 absmax target

_prog_cache = {}


def _build_program(C):
    import concourse.mybir as mybir
    import concourse.tile as tile
    from concourse import bacc

    f32 = mybir.dt.float32
    bf16 = mybir.dt.bfloat16
    e3 = mybir.dt.float8e3
    Act = mybir.ActivationFunctionType

    blocks = C // CB
    nblk = EPC * blocks            # token blocks per core
    nquad = (nblk + QUAD - 1) // QUAD
    assert nblk % QUAD == 0, "token blocks must tile into quads"

    nc = bacc.Bacc("TRN2", target_bir_lowering=False, debug=False)

    # W1 and W2 are separate tensors so fc1 weights stream first and all
    # fc1 compute overlaps the tail of the W2 stream, all on the SP ring.
    # W1 ships as 2.1MB quad chunks; W2 as two global hc-half chunks (all
    # experts x hc 0-3, then hc 4-7) so fc2's first half runs while the
    # last chunk streams. All chunks use 16KB descriptors.
    wq1 = nc.dram_tensor("wq1", [EPC // 4, 128, 4 * 4096], e3,
                         kind="ExternalInput")
    wq2a = nc.dram_tensor("wq2a", [128, EPC * 2048], e3,
                          kind="ExternalInput")
    wq2b = nc.dram_tensor("wq2b", [EPC // 4, 128, 4 * 2048], e3,
                          kind="ExternalInput")
    xt = nc.dram_tensor("xt", [128, nblk * DC * CB], bf16, kind="ExternalInput")
    idt = nc.dram_tensor("idt", [128, 128], bf16, kind="ExternalInput")
    yt = nc.dram_tensor("yt", [nquad, 128, D], bf16, kind="ExternalOutput")

    with tile.TileContext(nc) as tc:
        with (
            tc.tile_pool(name="w1pool", bufs=2) as w1pool,
            tc.tile_pool(name="w2pool", bufs=2) as w2pool,
            tc.tile_pool(name="xpool", bufs=1) as xpool,
            tc.tile_pool(name="cpool", bufs=1) as cpool,
            tc.tile_pool(name="hpool", bufs=2) as hpool,
            tc.tile_pool(name="ypool", bufs=2) as ypool,
            tc.tile_pool(name="psh", bufs=2, space="PSUM") as pshp,
            tc.tile_pool(name="pst", bufs=2, space="PSUM") as pstp,
            tc.tile_pool(name="psy", bufs=2, space="PSUM") as psyp,
        ):
            # ident + x ride the ACT HWDGE ring; all weight DMAs go on the
            # SP ring, which runs no compute so issues never stall
            ident = cpool.tile([128, 128], bf16)
            nc.scalar.dma_start(ident[:], idt[:])
            xall = xpool.tile([128, nblk * DC * CB], bf16)
            nc.scalar.dma_start(xall[:], xt[:])

            # warm the ACT silu table off the critical path
            warm = cpool.tile([128, 1], bf16)
            nc.gpsimd.memset(warm[:], 0.0)
            nc.scalar.activation(warm[:], warm[:], Act.Silu)

            w1_of = {}           # expert slot -> fc1 weight tile (fp8e3)
            w2_of = {}           # expert slot -> fc2 weight tile

            def load_w1(s):
                # one DMA covers experts [4g, 4g+4)
                g = s // 4
                w1 = w1pool.tile([128, 4 * 4096], e3, tag="w1")
                nc.sync.dma_start(w1[:], wq1[g])
                for k in range(4):
                    w1_of[4 * g + k] = w1[:, k * 4096:(k + 1) * 4096]

            def load_w2_all():
                # hc 0-3 for all experts in one chunk, then hc 4-7 in
                # per-group chunks so the last arrival gates only 16 MMs.
                # w2_of maps (expert, hc-half) -> 2048-col slice.
                # w2a rides the otherwise-idle ACT ring: the shared SDMA
                # engines fill the SP ring's per-DMA handoff gaps with it
                w2a = w2pool.tile([128, EPC * 2048], e3, tag="w2a")
                nc.scalar.dma_start(w2a[:], wq2a[:])
                for k in range(EPC):
                    w2_of[(k, 0)] = w2a[:, k * 2048:(k + 1) * 2048]
                for g in range(EPC // 4):
                    w2b = w2pool.tile([128, 4 * 2048], e3, tag="w2b")
                    nc.sync.dma_start(w2b[:], wq2b[g])
                    for k in range(4):
                        w2_of[(4 * g + k, 1)] = \
                            w2b[:, k * 2048:(k + 1) * 2048]

            # token block tb = (expert slot s, block b); x block index
            tbs = [(s, b) for s in range(EPC) for b in range(blocks)]

            def emit_fc1(q, qtbs):
                # psh[32j+i, h] = sum_d xs[d, tok i of tb j] * W1T[d, h]
                psh = pshp.tile([128, H], f32, tag="psh")
                for dc in range(DC):
                    for nh in range(2):
                        for j, (s, b) in enumerate(qtbs):
                            xoff = ((s * blocks + b) * DC + dc) * CB
                            nc.tensor.matmul(
                                psh[32 * j:32 * (j + 1),
                                    nh * 512:(nh + 1) * 512],
                                xall[:, xoff:xoff + CB],
                                w1_of[s][:, dc * H + nh * 512:
                                         dc * H + (nh + 1) * 512],
                                start=(dc == 0),
                                stop=(dc == DC - 1),
                                tile_position=(0, 32 * j),
                            )
                # silu on the whole quad -> bf16
                hq = hpool.tile([128, H], bf16, tag="hq")
                nc.scalar.activation(hq[:], psh[:], Act.Silu)
                return hq

            def emit_trans(hq):
                # transpose: hq [tokq, h] -> htq [h, tokq], 8 chunks of 128
                pst = pstp.tile([128, H], bf16, tag="pst")
                for hc in range(HC):
                    nc.tensor.transpose(
                        pst[:, hc * 128:(hc + 1) * 128],
                        hq[:, hc * 128:(hc + 1) * 128],
                        ident[:],
                    )
                htq = hpool.tile([128, H], bf16, tag="htq")
                nc.vector.tensor_copy(htq[:], pst[:])
                return htq

            def emit_fc2_half(qtbs, htq, psy, hcs):
                # fc2: psy[32j+i, d] = sum_h htq[h, 32j+i] * W2T[h, d]
                for hc in hcs:
                    for j, (s, b) in enumerate(qtbs):
                        w2sl = w2_of[(s, hc // 4)]
                        nc.tensor.matmul(
                            psy[32 * j:32 * (j + 1), :],
                            htq[:, hc * 128 + 32 * j:hc * 128 + 32 * (j + 1)],
                            w2sl[:, (hc % 4) * D:(hc % 4 + 1) * D],
                            start=(hc == 0),
                            stop=(hc == HC - 1),
                            tile_position=(0, 32 * j),
                        )

            def emit_y(q, psy):
                yq = ypool.tile([128, D], bf16, tag="yq")
                nc.vector.tensor_copy(yq[:], psy[:])
                nc.sync.dma_start(yt[q], yq[:])

            def emit_fc2(q, qtbs, hq):
                htq = emit_trans(hq)
                psy = psyp.tile([128, D], f32, tag="psy")
                emit_fc2_half(qtbs, htq, psy, range(HC))
                emit_y(q, psy)

            if nquad <= 2:
                # cross-quad pipeline: all fc1 phases first (stream order),
                # then transposes, then fc2 in two hc-phases across BOTH
                # quads so phase A overlaps the last W2 chunk's stream and
                # neither quad's ready work is stuck behind the other
                hqs = []
                for q in range(nquad):
                    qtbs = tbs[q * QUAD:(q + 1) * QUAD]
                    for s, b in qtbs:
                        if s not in w1_of:
                            load_w1(s)
                    hqs.append(emit_fc1(q, qtbs))
                load_w2_all()
                htqs = []
                psys = []
                for q in range(nquad):
                    htq = emit_trans(hqs[q])
                    htqs.append(htq)
                    psy = psyp.tile([128, D], f32, tag="psy")
                    psys.append(psy)
                for q in range(nquad):
                    emit_fc2_half(tbs[q * QUAD:(q + 1) * QUAD],
                                  htqs[q], psys[q], range(0, HC // 2))
                for q in range(nquad):
                    emit_fc2_half(tbs[q * QUAD:(q + 1) * QUAD],
                                  htqs[q], psys[q], range(HC // 2, HC))
                    emit_y(q, psys[q])
            else:
                loaded = [False]
                for q in range(nquad):
                    qtbs = tbs[q * QUAD:(q + 1) * QUAD]
                    for s, b in qtbs:
                        if s not in w1_of:
                            load_w1(s)
                    hq = emit_fc1(q, qtbs)
                    if not loaded[0]:
                        load_w2_all()
                        loaded[0] = True
                    emit_fc2(q, qtbs, hq)

    nc.compile()
    return nc


def _route(expert_idx):
    idx = np.asarray(expert_idx).astype(np.int64)
    order = np.argsort(idx, kind="stable")
    counts = np.bincount(idx, minlength=E)
    starts = np.zeros(E + 1, dtype=np.int64)
    starts[1:] = np.cumsum(counts)
    return order, starts, counts


def _pack_inputs(x, fc1_w, fc2_w, order, starts, C):
    import ml_dtypes

    bf16 = ml_dtypes.bfloat16
    e3 = ml_dtypes.float8_e3m4
    blocks = C // CB
    nblk = EPC * blocks

    # per-channel scales: s1[e, d] (folded into x packing), s2[e, d]
    # (folded into host output unpacking)
    s1 = np.abs(fc1_w).max(axis=1) / FP8_MAX + 1e-30     # [E, D]
    s2 = np.abs(fc2_w).max(axis=2) / FP8_MAX + 1e-30     # [E, D]
    _unpack_outputs._s2 = s2

    in_maps = []
    for core in range(NCORES):
        wh1 = np.empty((EPC // 4, 4, 128, 4096), e3)
        wh2 = np.empty((2, EPC, 128, 2048), e3)
        wh2b = wh2[1].reshape(EPC // 4, 4, 128, 2048)
        xh = np.zeros((128, nblk * DC * CB), np.float32)
        for s in range(EPC):
            e = core * EPC + s
            # W1T cols: col dc*H + h = W1[h, dc*128+p]; scale s1[e, dc*128+p]
            w1t = np.ascontiguousarray(fc1_w[e].T).reshape(DC, 128, H)
            w1c = w1t.transpose(1, 0, 2).reshape(128, DC * H)
            sc1 = np.repeat(s1[e].reshape(DC, 128).T[:, :, None], H, axis=2)
            # W2T cols: col hc*D + d = W2[d, hc*128+p]; scale s2[e, d]
            w2t = np.ascontiguousarray(fc2_w[e].T).reshape(HC, 128, D)
            w2c = w2t.transpose(1, 0, 2).reshape(128, HC * D)
            sc2 = np.broadcast_to(s2[e][None, None, :], (128, HC, D))
            w1s = w1c / sc1.reshape(128, DC * H)
            w2s = (w2c / sc2.reshape(128, HC * D)).astype(e3)
            wh1[s // 4, s % 4] = w1s.astype(e3)
            wh2[0, s] = w2s[:, :2048]
            wh2[1, s] = w2s[:, 2048:]

            toks = order[starts[e]:starts[e + 1]]
            n = len(toks)
            if n:
                xs = x[toks] * s1[e][None, :]            # fold W1 scales
                xte = np.ascontiguousarray(xs.T).reshape(DC, 128, n)
                for b in range(blocks):
                    lo, hi = b * CB, min(n, (b + 1) * CB)
                    if lo >= hi:
                        break
                    for c in range(DC):
                        base = ((s * blocks + b) * DC + c) * CB
                        xh[:, base:base + hi - lo] = xte[c][:, lo:hi]
        in_maps.append({
            "wq1": wh1.transpose(0, 2, 1, 3).reshape(EPC // 4, 128, 4 * 4096),
            "wq2a": wh2[0].transpose(1, 0, 2).reshape(128, EPC * 2048),
            "wq2b": wh2b.transpose(0, 2, 1, 3).reshape(EPC // 4, 128,
                                                       4 * 2048),
            "xt": xh.astype(bf16),
            "idt": np.eye(128, dtype=np.float32).astype(bf16),
        })
    return in_maps


def _unpack_outputs(results, order, starts, C, out_dtype):
    fc2_scale = _unpack_outputs._s2
    blocks = C // CB
    out = np.zeros((T, D), out_dtype)
    for core in range(NCORES):
        yh = np.asarray(results[core]["yt"], np.float32)   # [nquad, 128, D]
        yh = yh.reshape(-1, CB, D)                         # [nblk, CB, D]
        for s in range(EPC):
            e = core * EPC + s
            toks = order[starts[e]:starts[e + 1]]
            n = len(toks)
            for b in range(blocks):
                lo, hi = b * CB, min(n, (b + 1) * CB)
                if lo >= hi:
                    break
                out[toks[lo:hi]] = (yh[s * blocks + b, :hi - lo]
                                    * fc2_scale[e][None, :])
    return out


def kernel(x, expert_idx, fc1_w, fc2_w):
    from concourse.bass_utils import run_bass_kernel_spmd

    x = np.asarray(x, dtype=np.float32)
    fc1_w = np.asarray(fc1_w, dtype=np.float32)
    fc2_w = np.asarray(fc2_w, dtype=np.float32)

    order, starts, counts = _route(expert_idx)
    C = max(CB, int(-(-int(counts.max()) // CB) * CB))

    if C not in _prog_cache:
        _prog_cache[C] = _build_program(C)
    nc = _prog_cache[C]

    in_maps = _pack_inputs(x, fc1_w, fc2_w, order, starts, C)
    res = run_bass_kernel_spmd(nc, in_maps, list(range(NCORES)))
    return _unpack_outputs(res.results, order, starts, C, np.float32)
